# revision 11
# baseline (speedup 1.0000x reference)
"""Trainium2 Bass kernel for nn_ActDistillLoss (L=12, N=1024, D=2048, A=7).

Distribution (8 cores, SPMD — one program, per-core input slices):
  - layers 0..7 -> core c owns layer c fully ("A": full upper-triangle gram)
  - layers 8..11 -> split between core pairs ("B"): the 8x8 block-pair set
    {(i,j): i<=j} is partitioned into two isomorphic halves T and sigma(T)
    (sigma = rotate block labels by one); core h receives the layer with rows
    rotated by 128*h so the same instruction stream computes its half.
  Per core:
    * normalize rows (ACT square-accum -> sqrt -> DVE reciprocal; GPSIMD
      scale to bf16), PE-transpose normalized rows into [D, N] bf16 layout
    * gram tiles (bf16 matmul, fp32 PSUM) only for needed block pairs;
      (Ms-Mt)^2 via DVE sub + ACT Square(scale=sqrt(w))+accum per segment
    * align loss: fp32 row dots (GPSIMD mult + DVE reduce) * rsqrt norms
    * action-loss partial SSEs
  Output: [128, 64] fp32 partial-sum accumulator per core; host applies the
  static layer weights / ALPHA / BETA / ETA and sums the 8 results.
"""

import sys

import numpy as np

if "/opt/trn_rl_repo" not in sys.path:
    sys.path.insert(0, "/opt/trn_rl_repo")

import concourse.mybir as mybir  # noqa: E402
from concourse import bacc  # noqa: E402
from concourse.tile import TileContext  # noqa: E402

FP32 = mybir.dt.float32
BF16 = mybir.dt.bfloat16
ALU = mybir.AluOpType
ACTF = mybir.ActivationFunctionType
AXL = mybir.AxisListType

# problem constants (hardcoded per spec)
L, N_FULL, D_FULL, A_DIM = 12, 1024, 2048, 7
NCORES = 8
ALPHA, BETA, ETA, GAMMA = 1.0, 1.0, 0.5, 2.0
SQRT2 = float(np.sqrt(2.0))

P = 128
ACC_W = 64


def _b_arcs(NP):
    """Half-triangle pair pattern T: i even, blocks [(i+d)%NP, d<NP/2] plus
    the {i,i+NP/2} extras for even i < NP/2. T and its rotate-by-1 image
    partition the full {(i,j): i<=j} block-pair set (NP >= 4, even)."""
    arcs = []
    for i in range(0, NP, 2):
        njs = NP // 2 + (1 if i < NP // 2 else 0)
        b0, b1 = i, i + njs
        runs = [(b0, b1)] if b1 <= NP else [(b0, NP), (0, b1 - NP)]
        arcs.append((i, runs))
    return arcs


def layer_tasks(NP, kind):
    """-> [(i, c0_blk, c1_blk, [(s0_blk, s1_blk, w), ...])] gram tiles."""
    if kind == "a":
        arcs = [(i, [(i, NP)]) for i in range(NP)]
    else:
        arcs = _b_arcs(NP)
    tasks = []
    for i, runs in arcs:
        for b0, b1 in runs:
            c = b0
            while c < b1:
                ce = min(c + 4, b1)
                segs = []
                for j in range(c, ce):
                    w = 1 if j == i else 2
                    if segs and segs[-1][2] == w:
                        segs[-1] = (segs[-1][0], j + 1, w)
                    else:
                        segs.append((j, j + 1, w))
                tasks.append((i, c, ce, segs))
                c = ce
    return tasks


class Plan:
    def __init__(self, N=N_FULL, D=D_FULL, a_dim=A_DIM):
        self.N, self.D, self.a_dim = N, D, a_dim
        self.NP = N // P
        self.KC = D // P
        self.tasks = {"a": layer_tasks(self.NP, "a"), "b": layer_tasks(self.NP, "b")}
        self.own_rb = {"a": list(range(self.NP)), "b": list(range(0, self.NP, 2))}
        col = 0
        self.cos_cols = {}
        for ln in ("a", "b"):
            self.cos_cols[ln] = list(range(col, col + len(self.own_rb[ln])))
            col += len(self.own_rb[ln])
        self.struct_cols = {}
        for ln in ("a", "b"):
            cols = []
            for _i, _c0, _c1, segs in self.tasks[ln]:
                for _ in segs:
                    cols.append(col)
                    col += 1
            self.struct_cols[ln] = cols
        self.act_cols = {}
        for ln in ("a", "b"):
            self.act_cols[ln] = list(range(col, col + 3))
            col += 3
        assert col <= ACC_W, col
        self.ncols = col


def build_program(N=N_FULL, D=D_FULL, a_dim=A_DIM, num_devices=NCORES):
    pl = Plan(N, D, a_dim)
    NP, KC = pl.NP, pl.KC
    TB = min(8, KC)  # transposes per PSUM tile
    AF = {"a": NP * a_dim, "b": (NP // 2) * a_dim}

    nc = bacc.Bacc(
        "TRN2", target_bir_lowering=False, debug=False, num_devices=num_devices
    )

    sem = {}
    for ln in ("a", "b"):
        for t in ("stu", "tea"):
            sem[(ln, t)] = nc.dram_tensor(
                f"sem_{ln}_{t}", [N, D], FP32, kind="ExternalInput"
            )
    act = {}
    for ln in ("a", "b"):
        for t in ("stu", "tea", "prev", "gt"):
            act[(ln, t)] = nc.dram_tensor(
                f"act_{ln}_{t}", [P, AF[ln]], FP32, kind="ExternalInput"
            )
    ident_dram = nc.dram_tensor("ident", [P, P], BF16, kind="ExternalInput")
    out = nc.dram_tensor("out", [P, ACC_W], FP32, kind="ExternalOutput")

    with TileContext(nc) as tc:
        with (
            tc.tile_pool(name="const", bufs=1) as const_pool,
            tc.tile_pool(name="accp", bufs=1) as acc_pool,
            tc.tile_pool(name="raw", bufs=3) as raw_pool,
            tc.tile_pool(name="sca", bufs=3) as sca_pool,
            tc.tile_pool(name="snt", bufs=2) as snt_pool,
            tc.tile_pool(name="dump", bufs=2) as dump_pool,
            tc.tile_pool(name="prod", bufs=2) as prod_pool,
            tc.tile_pool(name="gdiff", bufs=2) as gdiff_pool,
            tc.tile_pool(name="small", bufs=6) as small_pool,
            tc.tile_pool(name="actp", bufs=5) as act_pool,
            tc.tile_pool(name="ptr", bufs=2, space="PSUM") as ptr_pool,
            tc.tile_pool(name="pgram", bufs=5, space="PSUM") as pgram_pool,
        ):
            ident = const_pool.tile([P, P], BF16)
            nc.sync.dma_start(ident[:], ident_dram[:])
            accs = acc_pool.tile([P, ACC_W], FP32)
            nc.vector.memset(accs[:], 0.0)

            for ln in ("a", "b"):
                snt = {
                    t: snt_pool.tile([P, KC, N], BF16, tag=f"snt_{t}", name=f"snt_{t}")
                    for t in ("stu", "tea")
                }
                own = set(pl.own_rb[ln])
                for rb in range(NP):
                    raw = {}
                    rcp = {}
                    for t in ("stu", "tea"):
                        r = raw_pool.tile([P, D], FP32, tag="raw", name="raw")
                        nc.sync.dma_start(r[:], sem[(ln, t)][rb * P : (rb + 1) * P, :])
                        raw[t] = r
                        ss = small_pool.tile([P, 1], FP32, tag="ss", name="ss")
                        dmp = dump_pool.tile(
                            [P, max(D, 512)], FP32, tag="dump", name="dump"
                        )
                        nc.scalar.activation(
                            dmp[:, :D], r[:], ACTF.Square, accum_out=ss[:]
                        )
                        nrm = small_pool.tile([P, 1], FP32, tag="nrm", name="nrm")
                        nc.scalar.sqrt(nrm[:], ss[:])
                        rc = small_pool.tile([P, 1], FP32, tag="rcp", name="rcp")
                        nc.vector.reciprocal(rc[:], nrm[:])
                        rcp[t] = rc
                        s = sca_pool.tile([P, D], BF16, tag="sca", name="sca")
                        nc.gpsimd.tensor_scalar_mul(s[:], r[:], rc[:])
                        # transpose normalized bf16 rows into [D, N] layout
                        for q in range(KC // TB):
                            pt = ptr_pool.tile([P, TB * P], BF16, tag="ptr", name="ptr")
                            for j in range(TB):
                                k = q * TB + j
                                nc.tensor.transpose(
                                    pt[:, j * P : (j + 1) * P],
                                    s[:, k * P : (k + 1) * P],
                                    ident[:],
                                )
                            nc.vector.tensor_copy(
                                out=snt[t][
                                    :, q * TB : (q + 1) * TB, rb * P : (rb + 1) * P
                                ],
                                in_=pt[:].rearrange("p (t q) -> p t q", t=TB),
                            )
                    if rb in own:
                        # align: cos = <stu,tea> * rcp_s * rcp_t (fp32 raw dots)
                        pr = prod_pool.tile([P, D], BF16, tag="prod", name="prod")
                        nc.gpsimd.tensor_tensor(
                            pr[:], raw["stu"][:], raw["tea"][:], op=ALU.mult
                        )
                        dot = small_pool.tile([P, 1], FP32, tag="dot", name="dot")
                        nc.vector.tensor_reduce(dot[:], pr[:], axis=AXL.X, op=ALU.add)
                        col = pl.cos_cols[ln][pl.own_rb[ln].index(rb)]
                        nc.vector.scalar_tensor_tensor(
                            out=accs[:, col : col + 1],
                            in0=dot[:],
                            scalar=rcp["stu"][:],
                            in1=rcp["tea"][:],
                            op0=ALU.mult,
                            op1=ALU.mult,
                        )

                # gram tiles + struct epilogue
                ci = iter(pl.struct_cols[ln])
                for i, c0b, c1b, segs in pl.tasks[ln]:
                    w_cols = c1b * P - c0b * P
                    ps = {}
                    for t in ("stu", "tea"):
                        pg = pgram_pool.tile([P, 512], FP32, tag="pgram", name="pgram")
                        for k in range(KC):
                            nc.tensor.matmul(
                                pg[:, :w_cols],
                                snt[t][:, k, i * P : (i + 1) * P],
                                snt[t][:, k, c0b * P : c1b * P],
                                start=(k == 0),
                                stop=(k == KC - 1),
                            )
                        ps[t] = pg
                    mt_s = gdiff_pool.tile([P, 512], FP32, tag="mts", name="mts")
                    nc.vector.tensor_copy(out=mt_s[:, :w_cols], in_=ps["tea"][:, :w_cols])
                    gd = gdiff_pool.tile([P, 512], FP32, tag="gdiff", name="gdiff")
                    nc.vector.tensor_sub(
                        gd[:, :w_cols], ps["stu"][:, :w_cols], mt_s[:, :w_cols]
                    )
                    for s0b, s1b, w in segs:
                        col = next(ci)
                        r0, r1 = (s0b - c0b) * P, (s1b - c0b) * P
                        sd = dump_pool.tile(
                            [P, max(D, 512)], FP32, tag="dump", name="dump"
                        )
                        nc.scalar.activation(
                            sd[:, : r1 - r0],
                            gd[:, r0:r1],
                            ACTF.Square,
                            scale=(SQRT2 if w == 2 else 1.0),
                            accum_out=accs[:, col : col + 1],
                        )

            # action losses
            for ln in ("a", "b"):
                af = AF[ln]
                at = {}
                for t in ("stu", "tea", "prev", "gt"):
                    tl = act_pool.tile([P, af], FP32, tag=f"act_{ln}", name=f"act_{ln}")
                    nc.sync.dma_start(tl[:], act[(ln, t)][:])
                    at[t] = tl
                for j, other in enumerate(("gt", "tea", "prev")):
                    d = act_pool.tile([P, af], FP32, tag=f"actd_{ln}", name=f"actd_{ln}")
                    nc.gpsimd.tensor_tensor(
                        d[:], at["stu"][:], at[other][:], op=ALU.subtract
                    )
                    col = pl.act_cols[ln][j]
                    d2 = act_pool.tile(
                        [P, af], FP32, tag=f"actd2_{ln}", name=f"actd2_{ln}"
                    )
                    nc.scalar.activation(
                        d2[:], d[:], ACTF.Square, accum_out=accs[:, col : col + 1]
                    )

            nc.sync.dma_start(out[:], accs[:])

    nc.compile()
    return nc, pl


def make_in_maps(stu_sem, tea_sem, stu_act, tea_act, action_gt, N=N_FULL):
    stu_sem = np.ascontiguousarray(stu_sem, dtype=np.float32)
    tea_sem = np.ascontiguousarray(tea_sem, dtype=np.float32)
    stu_act = np.asarray(stu_act, dtype=np.float32)
    tea_act = np.asarray(tea_act, dtype=np.float32)
    action_gt = np.asarray(action_gt, dtype=np.float32)
    a_dim = stu_act.shape[2]
    NP = N // P

    prev_act = np.concatenate([np.zeros_like(stu_act[:1]), stu_act[:-1]], axis=0)
    ident = np.eye(P, dtype=np.float32)

    def act_rows(x, rows):
        o = rows.shape[0] // P
        return np.ascontiguousarray(
            x[rows].reshape(o, P, a_dim).transpose(1, 0, 2).reshape(P, o * a_dim)
        )

    in_maps = []
    for c in range(NCORES):
        la = c
        lb = 8 + c // 2
        h = c % 2
        rows_b = (np.arange(N) + P * h) % N  # rotate by one block for half 1
        own_b = np.arange(N).reshape(NP, P)[h::2].ravel()
        rows_a = np.arange(N)
        m = {
            "ident": ident,
            "sem_a_stu": np.ascontiguousarray(stu_sem[la]),
            "sem_a_tea": np.ascontiguousarray(tea_sem[la]),
            "sem_b_stu": np.ascontiguousarray(stu_sem[lb][rows_b]),
            "sem_b_tea": np.ascontiguousarray(tea_sem[lb][rows_b]),
            "act_a_stu": act_rows(stu_act[la], rows_a),
            "act_a_tea": act_rows(tea_act[la], rows_a),
            "act_a_prev": act_rows(prev_act[la], rows_a),
            "act_a_gt": act_rows(action_gt, rows_a),
            "act_b_stu": act_rows(stu_act[lb], own_b),
            "act_b_tea": act_rows(tea_act[lb], own_b),
            "act_b_prev": act_rows(prev_act[lb], own_b),
            "act_b_gt": act_rows(action_gt, own_b),
        }
        in_maps.append(m)
    return in_maps


def combine(results, pl=None, n_layers=L):
    if pl is None:
        pl = Plan()
    N, a_dim = pl.N, pl.a_dim
    w = (np.arange(n_layers, dtype=np.float64) / (n_layers - 1)) ** GAMMA
    w = w / w.mean()

    cos_sum = np.zeros(n_layers)
    struct_sum = np.zeros(n_layers)
    act_sse = np.zeros(n_layers)
    for c in range(NCORES):
        acc = np.asarray(results[c]["out"], dtype=np.float64)
        la = c
        lb = 8 + c // 2
        cos_sum[la] += acc[:, pl.cos_cols["a"]].sum()
        cos_sum[lb] += acc[:, pl.cos_cols["b"]].sum()
        struct_sum[la] += acc[:, pl.struct_cols["a"]].sum()
        struct_sum[lb] += acc[:, pl.struct_cols["b"]].sum()
        act_sse[la] += acc[:, pl.act_cols["a"]].sum()
        act_sse[lb] += acc[:, pl.act_cols["b"]].sum()

    l_align = 1.0 - cos_sum / N
    l_struct = struct_sum / (N * N)
    l_act = act_sse / (N * a_dim)
    total = np.sum(w * (ALPHA * (l_align + ETA * l_struct) + BETA * l_act))
    return np.float32(total)


_PROGRAM_CACHE = {}


def get_program():
    if "full" not in _PROGRAM_CACHE:
        _PROGRAM_CACHE["full"] = build_program()
    return _PROGRAM_CACHE["full"]


def kernel(stu_sem, tea_sem, stu_act, tea_act, action_gt):
    from concourse import bass_utils

    nc, pl = get_program()
    in_maps = make_in_maps(stu_sem, tea_sem, stu_act, tea_act, action_gt)
    res = bass_utils.run_bass_kernel_spmd(nc, in_maps, core_ids=list(range(NCORES)))
    return combine(res.results, pl)
